# revision 4
# baseline (speedup 1.0000x reference)
"""Trainium2 Bass kernel for nn_Attention_47459388621522.

Computation (B=256, N=2048, D=256):
    hidden = concat([feature, broadcast(pointer_hidden_state)], -1)   # [B,N,2D]
    pre    = tanh(einsum('de,bne->bnd', W[0], hidden))                # [B,N,D]
    scores = einsum('d,bnd->bn', v[0,0], pre)                         # [B,N]
    attns  = softmax(scores, axis=1)[:, None, :]                      # [B,1,N]

Split W = [Wf | Wh] along e: pre = tanh(feature @ Wf^T + bias_b) with
bias = pointer_hidden_state @ Wh^T computed on-device in exact fp32 (tiny).

Sharding: data-parallel over batch, 32 batches per core x 8 cores.

Host prep: feature is transposed per core to a [D, B_PER*N] fp32 "global
token stream" (contraction dim e lands on SBUF partitions; each partition's
DMA row is a long contiguous run). Matmuls run in float32r — fp32 bits with
the PE rounding inputs to 12 mantissa bits at 1 cycle/row, 4x the exact-fp32
rate. Measured final absmax vs the fp32 reference: 4.7e-3 (output scale 1.0).
An fp16 path (feature cast fp16, W sent as an fp16 hi+lo pair, absmax 5.9e-3)
is kept behind DTYPE="f16"; it halves DMA bytes but doubles PE work, which
measures slower because the kernel sits at the DMA/PE ridge.

Per-core dataflow (ch_tok tokens per DMA chunk, groups of 512 tokens,
d in 2 chunks of 128):
    PE : pre[d,t]   = sum_ko WfT[e,d]^T @ featT[e,t]   (2 f32r MMs, psum accum)
    ACT: th[d,t]    = tanh(pre + bias[d,b])            (per-partition bias)
    PE : sc[1,t]    = v[d,1]^T @ th[d,t]               (2 f32r MMs)
    DVE: sc -> stage[1, N] (psum->sbuf), gpsimd DMA -> scores rows
    softmax over [16, 2048] halves (DVE reduce-max, ACT exp(x-max)+accum via
    per-partition bias, DVE reciprocal+scale); half 0 overlaps the main loop.

Measured on 8 axon-tunneled trn2 NeuronCores (dispatch-floor-cancelling
repeat-slope method): 222.9 us/core device time vs a measured ~197 us
DMA-only floor (64 MB/core at ~340 GB/s); TimelineSim predicts 223.0 us.
"""

import numpy as np

import concourse.bacc as bacc
import concourse.mybir as mybir
import concourse.tile as tile
from concourse.bass_utils import run_bass_kernel_spmd

f32 = mybir.dt.float32
f32r = mybir.dt.float32r
f16 = mybir.dt.float16

B, N, D = 256, 2048, 256
N_CORES = 8
B_PER = B // N_CORES          # 32 batches per core
TG = 512                      # token group (matmul moving free dim)
NG = N // TG                  # 4 groups per batch
P = 128
DC = D // P                   # 2 d-chunks
KC = D // P                   # 2 e-chunks
TOKS = B_PER * N              # tokens per core

DTYPE = "f32r"                # "f16" (W hi/lo pair) or "f32r"

_CACHED = {}


def _build(repeat=1, ft_bufs=3, ch_tok=2048, th_bufs=3, scps_bufs=3, stage_bufs=3,
           mmps_bufs=2, mode="full", ft_queues="s", dtype=None):
    # ch_tok: tokens per feature DMA chunk (multiple of N).
    # ft_queues: DMA channels for the feature load, round-robin over chunks.
    #            s=SP-HWDGE, a=ACT-HWDGE, p=Pool-SWDGE.
    dtype = dtype or DTYPE
    mm_dt = f16 if dtype == "f16" else f32r
    NW = 2 if dtype == "f16" else 1   # W terms (hi/lo pair for fp16)
    assert ch_tok % N == 0 and TOKS % ch_tok == 0
    bat_per_ch = ch_tok // N

    nc = bacc.Bacc("TRN2", target_bir_lowering=False, debug=False, name="ptrattn")
    featT = nc.dram_tensor("featT", [D, TOKS], mm_dt, kind="ExternalInput")
    hT = nc.dram_tensor("hT", [D, B_PER], f32, kind="ExternalInput")
    wfT = nc.dram_tensor("wfT", [NW, D, D], mm_dt, kind="ExternalInput")
    whT = nc.dram_tensor("whT", [D, D], f32, kind="ExternalInput")
    vv = nc.dram_tensor("vv", [D, 1], mm_dt, kind="ExternalInput")
    out = nc.dram_tensor("attns", [B_PER, N], f32, kind="ExternalOutput")

    act = mybir.ActivationFunctionType

    with tile.TileContext(nc) as tc:
        with tc.tile_pool(name="singles", bufs=1) as singles, \
             tc.tile_pool(name="feat", bufs=ft_bufs) as feat_pool, \
             tc.tile_pool(name="th", bufs=th_bufs) as th_pool, \
             tc.tile_pool(name="stage", bufs=stage_bufs) as stage_pool, \
             tc.tile_pool(name="soft", bufs=1) as soft_pool, \
             tc.tile_pool(name="mmps", bufs=mmps_bufs, space="PSUM") as mmps, \
             tc.tile_pool(name="scps", bufs=scps_bufs, space="PSUM") as scps:

            # ---- constants (bias inputs first so bias is ready earliest) ----
            wh_full = singles.tile([P, KC, D], f32)
            nc.sync.dma_start(wh_full, whT.rearrange("(ko p) d -> p ko d", p=P))
            hT_sb = singles.tile([P, KC, B_PER], f32)
            nc.sync.dma_start(hT_sb, hT.rearrange("(ko p) b -> p ko b", p=P))
            wf_sb = singles.tile([P, NW, KC, D], mm_dt)
            nc.sync.dma_start(
                wf_sb, wfT.ap().rearrange("w (ko p) d -> p w ko d", p=P))
            v_sb = singles.tile([P, DC, 1], mm_dt)
            nc.sync.dma_start(v_sb, vv.rearrange("(ko p) one -> p ko one", p=P))

            # ---- bias[b, d] = Wh @ h_b  (exact fp32, tiny) ----
            # own psum tag so the first main matmuls don't wait on its banks
            bias_sb = singles.tile([P, DC, B_PER], f32)
            for dc in range(DC):
                bias_ps = scps.tile([P, B_PER], f32, tag="sc", bufs=None)
                for ko in range(KC):
                    nc.tensor.matmul(
                        bias_ps,
                        wh_full[:, ko, dc * P:(dc + 1) * P],
                        hT_sb[:, ko, :],
                        start=(ko == 0), stop=(ko == KC - 1),
                    )
                nc.vector.tensor_copy(bias_sb[:, dc, :], bias_ps)

            # scores accumulators, two halves so softmax(half0) overlaps the
            # main loop (DVE ops need base-partition 0, so separate tiles)
            HB = B_PER // 2
            scores_half = [soft_pool.tile([HB, N], f32, name=f"scores{h}", tag=f"scores{h}")
                           for h in range(2)]

            def softmax_half(h):
                scores = scores_half[h]
                negmax = soft_pool.tile([HB, 1], f32, tag=f"negmax{h}")
                nc.vector.tensor_reduce(
                    negmax, scores, axis=mybir.AxisListType.X,
                    op=mybir.AluOpType.max, negate=True)
                # exp(score - max) fused via per-partition bias; the ACT exp
                # LUT underflows cleanly to 0 for very negative inputs
                # (probed down to -10000), so no clamp pass is needed
                probs = soft_pool.tile([HB, N], f32, tag=f"probs{h}")
                sumexp = soft_pool.tile([HB, 1], f32, tag=f"sumexp{h}")
                nc.scalar.activation(
                    probs, scores, act.Exp, bias=negmax, scale=1.0,
                    accum_out=sumexp)
                rcp = soft_pool.tile([HB, 1], f32, tag=f"rcp{h}")
                nc.vector.reciprocal(rcp, sumexp)
                nc.vector.tensor_scalar_mul(probs, probs, rcp)
                nc.gpsimd.dma_start(out.ap()[h * HB:(h + 1) * HB, :], probs)

            # ---- main loop over feature chunks ----
            qmap = {"s": nc.sync, "a": nc.scalar, "p": nc.gpsimd}
            featT_r = featT.rearrange("(ko p) t -> p ko t", p=P)
            for rep in range(repeat):
                for ch in range(TOKS // ch_tok):
                    ft = feat_pool.tile([P, KC, ch_tok], mm_dt, tag="ft")
                    eng = qmap[ft_queues[ch % len(ft_queues)]]
                    ft_src = featT_r[:, :, ch * ch_tok:(ch + 1) * ch_tok]
                    if ch == 0 and rep == 0:
                        # split the first load so the pipeline starts on the
                        # first quarter instead of waiting for the full chunk
                        q = ch_tok // 4
                        for s in range(4):
                            eng.dma_start(ft[:, :, s * q:(s + 1) * q],
                                          ft_src[:, :, s * q:(s + 1) * q])
                    else:
                        eng.dma_start(ft, ft_src)

                    for bl in range(bat_per_ch):
                        b = ch * bat_per_ch + bl
                        stage = stage_pool.tile([1, N], f32, tag="stage")
                        if mode == "dma_only":
                            nc.vector.tensor_copy(stage[:, 0:8], ft[0:1, 0, 0:8])
                            h, row = divmod(b, HB)
                            nc.gpsimd.dma_start(
                                scores_half[h][row:row + 1, 0:2], stage[:, 0:2])
                            if row == HB - 1:
                                softmax_half(h)
                            continue
                        for g in range(NG):
                            ts = slice(bl * N + g * TG, bl * N + (g + 1) * TG)
                            th = th_pool.tile([P, DC, TG], mm_dt, tag="th")
                            for dc in range(DC):
                                pre = mmps.tile([P, TG], f32, tag=f"pre{dc}")
                                first, last = (0, 0), (KC - 1, NW - 1)
                                for ko in range(KC):
                                    for w in range(NW):
                                        nc.tensor.matmul(
                                            pre,
                                            wf_sb[:, w, ko, dc * P:(dc + 1) * P],
                                            ft[:, ko, ts],
                                            start=((ko, w) == first),
                                            stop=((ko, w) == last),
                                        )
                                nc.scalar.activation(
                                    th[:, dc, :], pre, act.Tanh,
                                    bias=bias_sb[:, dc, b:b + 1], scale=1.0)
                            sc = scps.tile([1, TG], f32, tag="sc")
                            for dc in range(DC):
                                nc.tensor.matmul(
                                    sc, v_sb[:, dc, :], th[:, dc, :],
                                    start=(dc == 0), stop=(dc == DC - 1),
                                )
                            nc.vector.tensor_copy(stage[:, g * TG:(g + 1) * TG], sc)
                        # separate queue from the ft loads (no head-of-line block)
                        h, row = divmod(b, HB)
                        nc.gpsimd.dma_start(scores_half[h][row:row + 1, :], stage)
                        if row == HB - 1:
                            softmax_half(h)

    nc.compile()
    return nc


def _host_prep(feature, pointer_hidden_state, v, W, dtype=None):
    dtype = dtype or DTYPE
    Wf = W[0][:, :D]
    whT = np.ascontiguousarray(W[0][:, D:].T.astype(np.float32))   # [e, d]
    if dtype == "f16":
        np_dt = np.float16
        wfT32 = np.ascontiguousarray(Wf.T.astype(np.float32))      # [e, d]
        whi = wfT32.astype(np.float16)
        wlo = (wfT32 - whi.astype(np.float32)).astype(np.float16)
        wfT = np.stack([whi, wlo])                                  # [2, e, d]
    else:
        np_dt = np.float32
        wfT = np.ascontiguousarray(Wf.T.astype(np.float32))[None]  # [1, e, d]
    vv = np.ascontiguousarray(v[0, 0][:, None].astype(np_dt))      # [D, 1]
    per_core = []
    for c in range(N_CORES):
        sl = slice(c * B_PER, (c + 1) * B_PER)
        # [D, B_PER*N] global token stream: featT[e, b*N+n] = feature[b, n, e]
        featT = np.ascontiguousarray(
            feature[sl].astype(np_dt).transpose(2, 0, 1).reshape(D, TOKS))
        hT = np.ascontiguousarray(pointer_hidden_state[sl].T.astype(np.float32))
        per_core.append({"featT": featT, "hT": hT, "wfT": wfT, "whT": whT, "vv": vv})
    return per_core


def kernel(feature, pointer_hidden_state, v, W):
    feature = np.asarray(feature)
    pointer_hidden_state = np.asarray(pointer_hidden_state)
    v = np.asarray(v)
    W = np.asarray(W)

    if "nc" not in _CACHED:
        _CACHED["nc"] = _build()
    nc = _CACHED["nc"]

    in_maps = _host_prep(feature, pointer_hidden_state, v, W)
    res = run_bass_kernel_spmd(nc, in_maps, core_ids=list(range(N_CORES)))
    _CACHED["last_res"] = res
    outs = [res.results[c]["attns"] for c in range(N_CORES)]
    return np.concatenate(outs, axis=0)[:, None, :].astype(np.float32)



# revision 5
# speedup vs baseline: 1.4921x; 1.4921x over previous
"""Trainium2 Bass kernel for nn_Attention_47459388621522.

Computation (B=256, N=2048, D=256):
    hidden = concat([feature, broadcast(pointer_hidden_state)], -1)   # [B,N,2D]
    pre    = tanh(einsum('de,bne->bnd', W[0], hidden))                # [B,N,D]
    scores = einsum('d,bnd->bn', v[0,0], pre)                         # [B,N]
    attns  = softmax(scores, axis=1)[:, None, :]                      # [B,1,N]

Split W = [Wf | Wh] along e: pre = tanh(feature @ Wf^T + bias_b) with
bias = pointer_hidden_state @ Wh^T computed on-device in exact fp32 (tiny).

Sharding: data-parallel over batch, 32 batches per core x 8 cores.

v2/v3 design (f16): feature+Wf+v cast to fp16 on the host (halves the DMA
bytes vs the f32r baseline: 32 MB/core, ~98 us floor at ~340 GB/s).
Per-core loop processes one batch (2048 tokens) at a time:
    PE : pre[d,t] psum [128,1024] x2 per batch (8 MMs of 512 cols, f16)
    ACT: th[d,t] = tanh(pre + bias[d,b]) f16, FD=1024 per instr
    v-dot (VDOT mode):
      "stream": sc[1,512] = v^T @ th  (psum, 2 dc MMs), DVE copy to a
                stage row, gpsimd DMA into scores rows (f32r-baseline style)
      "col":    4 concurrent column-tiled MMs (tile_position=(0,32j)),
                stationary = zero-padded v at column b so batch b's scores
                land on psum partition 32j+b; one whole-bank DVE copy per
                16-batch half + one gather DMA -> scores_half [16, 2048]
    softmax per 16-batch half overlaps the main loop.
"""

import numpy as np

import concourse.bacc as bacc
import concourse.mybir as mybir
import concourse.tile as tile
from concourse.bass_utils import run_bass_kernel_spmd

f32 = mybir.dt.float32
f32r = mybir.dt.float32r
f16 = mybir.dt.float16

B, N, D = 256, 2048, 256
N_CORES = 8
B_PER = B // N_CORES          # 32 batches per core
TG = 1024                     # token group (ACT free dim; 2 psum banks)
NG = N // TG                  # 2 groups per batch
P = 128
DC = D // P                   # 2 d-chunks
KC = D // P                   # 2 e-chunks
TOKS = B_PER * N              # tokens per core
HB = B_PER // 2               # batches per scores half

VDOT = "col"                  # "col" (tile_position) or "stream"

_CACHED = {}


def _build(repeat=1, ft_bufs=3, ch_tok=2048, th_bufs=2, mmps_bufs=3,
           mode="full", ft_queues="sa", vdot=None):
    # ch_tok: tokens per feature DMA chunk (multiple of N).
    # ft_queues: DMA channels for the feature load, round-robin over chunks.
    #            s=SP-HWDGE, a=ACT-HWDGE, p=Pool-SWDGE.
    vdot = vdot or VDOT
    assert ch_tok % N == 0 and TOKS % ch_tok == 0
    bat_per_ch = ch_tok // N

    nc = bacc.Bacc("TRN2", target_bir_lowering=False, debug=False, name="ptrattn")
    featT = nc.dram_tensor("featT", [D, TOKS], f16, kind="ExternalInput")
    hT = nc.dram_tensor("hT", [D, B_PER], f32, kind="ExternalInput")
    wfT = nc.dram_tensor("wfT", [D, D], f16, kind="ExternalInput")
    whT = nc.dram_tensor("whT", [D, D], f32, kind="ExternalInput")
    vv = nc.dram_tensor("vv", [D, 1], f16, kind="ExternalInput")
    out = nc.dram_tensor("attns", [B_PER, N], f32, kind="ExternalOutput")

    act = mybir.ActivationFunctionType

    with tile.TileContext(nc) as tc:
        with tc.tile_pool(name="singles", bufs=1) as singles, \
             tc.tile_pool(name="feat", bufs=ft_bufs) as feat_pool, \
             tc.tile_pool(name="th", bufs=th_bufs) as th_pool, \
             tc.tile_pool(name="stage", bufs=3) as stage_pool, \
             tc.tile_pool(name="soft", bufs=1) as soft_pool, \
             tc.tile_pool(name="mmps", bufs=mmps_bufs, space="PSUM") as mmps, \
             tc.tile_pool(name="scps", bufs=1, space="PSUM") as scps:

            # ---- constants (bias inputs first so bias is ready earliest) ----
            wh_full = singles.tile([P, KC, D], f32)
            nc.sync.dma_start(wh_full, whT.rearrange("(ko p) d -> p ko d", p=P))
            hT_sb = singles.tile([P, KC, B_PER], f32)
            nc.sync.dma_start(hT_sb, hT.rearrange("(ko p) b -> p ko b", p=P))
            wf_sb = singles.tile([P, KC, D], f16)
            nc.sync.dma_start(wf_sb, wfT.rearrange("(ko p) d -> p ko d", p=P))
            # zero-padded v: vpad[:, dc, 0:32] = 0, vpad[:, dc, 32] = v chunk
            vpad = singles.tile([P, DC, 33], f16)
            nc.vector.memset(vpad, 0.0)
            nc.sync.dma_start(
                vpad[:, :, 32:33], vv.rearrange("(ko p) one -> p ko one", p=P))
            zpad = singles.tile([P, 512], f16)
            nc.vector.memset(zpad, 0.0)

            # ---- bias[b, d] = Wh @ h_b  (exact fp32, tiny) ----
            bias_sb = singles.tile([P, DC, B_PER], f32)
            for dc in range(DC):
                bias_ps = mmps.tile([P, TG], f32, tag="pre", bufs=None)
                for ko in range(KC):
                    nc.tensor.matmul(
                        bias_ps[:, :B_PER],
                        wh_full[:, ko, dc * P:(dc + 1) * P],
                        hT_sb[:, ko, :],
                        start=(ko == 0), stop=(ko == KC - 1),
                    )
                nc.vector.tensor_copy(bias_sb[:, dc, :], bias_ps[:, :B_PER])

            # scores accumulators, two halves so softmax(half0) overlaps the
            # main loop (DVE ops need base-partition 0, so separate tiles)
            scores_half = [soft_pool.tile([HB, N], f32, name=f"scores{h}", tag=f"scores{h}")
                           for h in range(2)]
            if vdot == "col":
                # psum score banks for "col" vdot: partition 32j+b, cols =
                # tokens of quarter j; one bank per 16-batch half
                sc_banks = [scps.tile([P, 512], f32, name=f"scb{h}", tag=f"scb{h}")
                            for h in range(2)]
                sc_sb = [stage_pool.tile([P, 512], f32, name=f"scsb{h}",
                                         tag=f"scsb{h}", bufs=1)
                         for h in range(2)]

            def softmax_half(h):
                scores = scores_half[h]
                negmax = soft_pool.tile([HB, 1], f32, tag=f"negmax{h}")
                nc.vector.tensor_reduce(
                    negmax, scores, axis=mybir.AxisListType.X,
                    op=mybir.AluOpType.max, negate=True)
                # exp(score - max) fused via per-partition bias; the ACT exp
                # LUT underflows cleanly to 0 for very negative inputs
                probs = soft_pool.tile([HB, N], f32, tag=f"probs{h}")
                sumexp = soft_pool.tile([HB, 1], f32, tag=f"sumexp{h}")
                nc.scalar.activation(
                    probs, scores, act.Exp, bias=negmax, scale=1.0,
                    accum_out=sumexp)
                rcp = soft_pool.tile([HB, 1], f32, tag=f"rcp{h}")
                nc.vector.reciprocal(rcp, sumexp)
                nc.vector.tensor_scalar_mul(probs, probs, rcp)
                nc.gpsimd.dma_start(out.ap()[h * HB:(h + 1) * HB, :], probs)

            # ---- main loop over feature chunks ----
            qmap = {"s": nc.sync, "a": nc.scalar, "p": nc.gpsimd}
            featT_r = featT.rearrange("(ko p) t -> p ko t", p=P)
            for rep in range(repeat):
                for ch in range(TOKS // ch_tok):
                    ft = feat_pool.tile([P, KC, ch_tok], f16, tag="ft")
                    eng = qmap[ft_queues[ch % len(ft_queues)]]
                    ft_src = featT_r[:, :, ch * ch_tok:(ch + 1) * ch_tok]
                    if ch == 0 and rep == 0:
                        # split the first load so the pipeline starts on the
                        # first quarter instead of waiting for the full chunk
                        q = ch_tok // 4
                        for s in range(4):
                            eng.dma_start(ft[:, :, s * q:(s + 1) * q],
                                          ft_src[:, :, s * q:(s + 1) * q])
                    else:
                        eng.dma_start(ft, ft_src)

                    for bl in range(bat_per_ch):
                        b = ch * bat_per_ch + bl
                        h, brow = divmod(b, HB)
                        if brow == 0 and vdot == "col":
                            # zero the whole score bank (start=True writes 0
                            # everywhere and sets has_written uniformly); all
                            # batch v-MMs below are then pure accumulates.
                            nc.tensor.matmul(
                                sc_banks[h], zpad[:, 0:128], zpad,
                                start=True, stop=False, skip_group_check=True)
                        if mode == "dma_only":
                            stage = stage_pool.tile([1, N], f32, tag="stage")
                            nc.vector.tensor_copy(stage[:, 0:8], ft[0:1, 0, 0:8])
                            nc.gpsimd.dma_start(
                                scores_half[h][brow:brow + 1, 0:2], stage[:, 0:2])
                            if brow == HB - 1:
                                softmax_half(h)
                            continue
                        th = th_pool.tile([P, DC, N], f16, tag="th")
                        for g in range(NG):
                            ts = slice(bl * N + g * TG, bl * N + (g + 1) * TG)
                            for dc in range(DC):
                                pre = mmps.tile([P, TG], f32, tag="pre")
                                for ko in range(KC):
                                    for half in range(TG // 512):
                                        cs = slice(half * 512, (half + 1) * 512)
                                        tsc = slice(ts.start + half * 512,
                                                    ts.start + (half + 1) * 512)
                                        nc.tensor.matmul(
                                            pre[:, cs],
                                            wf_sb[:, ko, dc * P:(dc + 1) * P],
                                            ft[:, ko, tsc],
                                            start=(ko == 0), stop=(ko == KC - 1),
                                        )
                                nc.scalar.activation(
                                    th[:, dc, g * TG:(g + 1) * TG], pre, act.Tanh,
                                    bias=bias_sb[:, dc, b:b + 1], scale=1.0)
                        if vdot == "col":
                            # scores for batch b: 4 column-tiled MMs per dc,
                            # concurrent across column groups j; batch lands
                            # on psum partition 32j + brow via the zero-pad
                            # trick (stationary [128, brow+1], v in last col)
                            for dc in range(DC):
                                for j in range(4):
                                    last = (brow == HB - 1 and dc == DC - 1
                                            and j == 3)
                                    nc.tensor.matmul(
                                        sc_banks[h][32 * j:32 * j + brow + 1, :],
                                        vpad[:, dc, 32 - brow:33],
                                        th[:, dc, 512 * j:512 * (j + 1)],
                                        start=False, stop=last,
                                        skip_group_check=True,
                                        tile_position=(0, 32 * j),
                                    )
                            if brow == HB - 1:
                                nc.vector.tensor_copy(sc_sb[h], sc_banks[h])
                                # gather [16, 2048]: batch row brow comes from
                                # partitions {32j+brow}, 512 cols each
                                for j in range(4):
                                    nc.gpsimd.dma_start(
                                        scores_half[h][:, 512 * j:512 * (j + 1)],
                                        sc_sb[h][32 * j:32 * j + HB, :])
                                softmax_half(h)
                        else:
                            stage = stage_pool.tile([1, N], f32, tag="stage")
                            for g2 in range(N // 512):
                                sc = scps.tile([1, 512], f32, tag="sc", bufs=2)
                                for dc in range(DC):
                                    nc.tensor.matmul(
                                        sc, vpad[:, dc, 32:33],
                                        th[:, dc, 512 * g2:512 * (g2 + 1)],
                                        start=(dc == 0), stop=(dc == DC - 1),
                                    )
                                nc.vector.tensor_copy(
                                    stage[:, 512 * g2:512 * (g2 + 1)], sc)
                            nc.gpsimd.dma_start(
                                scores_half[h][brow:brow + 1, :], stage)
                            if brow == HB - 1:
                                softmax_half(h)

    nc.compile()
    return nc


def _host_prep(feature, pointer_hidden_state, v, W):
    Wf = W[0][:, :D]
    whT = np.ascontiguousarray(W[0][:, D:].T.astype(np.float32))       # [e, d]
    wfT = np.ascontiguousarray(Wf.T.astype(np.float16))                # [e, d]
    vv = np.ascontiguousarray(v[0, 0][:, None].astype(np.float16))    # [D, 1]
    per_core = []
    for c in range(N_CORES):
        sl = slice(c * B_PER, (c + 1) * B_PER)
        # [D, B_PER*N] global token stream: featT[e, b*N+n] = feature[b, n, e]
        featT = np.ascontiguousarray(
            feature[sl].astype(np.float16).transpose(2, 0, 1).reshape(D, TOKS))
        hT = np.ascontiguousarray(pointer_hidden_state[sl].T.astype(np.float32))
        per_core.append({"featT": featT, "hT": hT, "wfT": wfT, "whT": whT, "vv": vv})
    return per_core


def kernel(feature, pointer_hidden_state, v, W):
    feature = np.asarray(feature)
    pointer_hidden_state = np.asarray(pointer_hidden_state)
    v = np.asarray(v)
    W = np.asarray(W)

    if "nc" not in _CACHED:
        _CACHED["nc"] = _build()
    nc = _CACHED["nc"]

    in_maps = _host_prep(feature, pointer_hidden_state, v, W)
    res = run_bass_kernel_spmd(nc, in_maps, core_ids=list(range(N_CORES)))
    _CACHED["last_res"] = res
    outs = [res.results[c]["attns"] for c in range(N_CORES)]
    return np.concatenate(outs, axis=0)[:, None, :].astype(np.float32)
